# revision 27
# baseline (speedup 1.0000x reference)
"""MoE feed-forward (top-2 of 8 experts) on 8 Trainium2 NeuronCores.

Strategy: expert-parallel. Core c owns expert c's weights; the gate +
top-2 routing runs on host in float64 (ordering verified robust vs the
f32 reference); each core computes bf16 GEMM1 -> SiLU -> GEMM2 with fp32
PSUM accumulation for the tokens routed to its expert, scales by the
combine weight on device, and the host sums each token's two expert rows.
The kernel is PE-bound: ~232 us of pure matmul at 78.6 TF/s bf16 (fp8
DoubleRow measured exactly 2x bf16 on HW, which is not enough to pay for
the hi/lo correction passes the 2e-2 absmax tolerance requires, so bf16
is optimal here).

Overhead engineering (the part that is actually tunable):
  - per-core capacity C = max expert load (2151, not 128-aligned): mm1
    cost is token-proportional and mm2 handles a partial final block, so
    no core pays for alignment padding; single weight set per core.
  - PE warm-up: 58 dummy 128-col matmuls bridge the ~14 us HBM latency of
    the first operands and hold the HAM clock at 2.4 GHz from the start.
  - k-packed DRAM layouts ([128, K, free]) so x / W1 / W2 arrive in ~20
    large strided DMAs in exact consumption order; the leading 384-token
    group burns slower than HBM feeds, so the ramp never stalls.
  - outputs in bf16 via the Scalar HWDGE queue (inputs own Sync), host
    sums in fp32.
  - software-pipelined groups: mm1(g0), mm1(g1), mm2(g0), ... keeps the
    PE fed across every mm1->mm2 boundary (99% issue efficiency mid-run).
"""

import numpy as np
import ml_dtypes

B, T, D, H, E = 4, 2048, 1024, 2048, 8
TOP_K = 2
N = B * T
P = 128
NCORES = 8
KD = D // P    # 8  contraction tiles for x @ W1
KH = H // P    # 16 contraction tiles for h @ W2
MM_FREE = 512  # moving-dim tile (PSUM bank limited for fp32 out)
WARMUP = 58

_compiled = {}


def _groups_of(c0, c1, first_small=False):
    """Chunk [c0, c1) into groups <= 512. The leading group is 384 tokens:
    small enough that its first operands land early, large enough that its
    PE burn rate stays below the HBM feed rate (no mid-ramp stalls)."""
    out = []
    lo = c0
    if first_small and c1 - lo >= 384 + 512:
        out.append((lo, 384))
        lo += 384
    while lo < c1:
        gs = min(MM_FREE, c1 - lo)
        out.append((lo, gs))
        lo += gs
    return out


def _build(C_A, C_B):
    import concourse.bacc as bacc
    import concourse.mybir as mybir
    import concourse.tile as tile

    fp32 = mybir.dt.float32
    bf16 = mybir.dt.bfloat16

    C = C_A + C_B
    n_sets = 2 if C_B else 1
    W1CH = 256            # w1 column chunk (one DMA): 0.5 MB, fine-grained
    NW1C = H // W1CH      # 8 w1 column chunks
    NW2Q = KH // 4        # 4 w2 k-quarters

    nc = bacc.Bacc("TRN2", target_bir_lowering=False, debug=False)

    # k-packed DRAM layouts: partition-major so big strided DMAs map 1:1
    # onto [128, k, cols] SBUF tiles.
    xT = nc.dram_tensor("xT", [P, KD, C], bf16, kind="ExternalInput").ap()
    w1d = [nc.dram_tensor(f"w1{s}", [P, KD, H], bf16, kind="ExternalInput").ap()
           for s in range(n_sets)]
    w2d = [nc.dram_tensor(f"w2{s}", [P, KH, D], bf16, kind="ExternalInput").ap()
           for s in range(n_sets)]
    NB = -(-C // P)       # combine-weight blocks (last may be partial)
    wt = nc.dram_tensor("wt", [P, NB], fp32, kind="ExternalInput").ap()
    out = nc.dram_tensor("out", [C, D], bf16, kind="ExternalOutput").ap()

    # token groups never straddle the primary/secondary boundary
    groups = [(g0, gs, 0) for g0, gs in _groups_of(0, C_A, first_small=True)]
    groups += [(g0, gs, 1) for g0, gs in _groups_of(C_A, C)]

    with tile.TileContext(nc) as tc:
        with (
            tc.tile_pool(name="persist", bufs=1) as persist,
            tc.tile_pool(name="xpool", bufs=4 if C_B else 6) as xpool,
            tc.tile_pool(name="hpool", bufs=2) as hpool,
            tc.tile_pool(name="opool", bufs=4) as opool,
            tc.tile_pool(name="psum1", bufs=4, space="PSUM") as psum1,
            tc.tile_pool(name="psum2", bufs=4, space="PSUM") as psum2,
        ):
            # --- PE warm-up on garbage SBUF: no DMA deps, ramps HAM while
            # the first operands stream in. Results are never read.
            # 128-col dummies: short enough that once real operands land the
            # tail-end waste is <200ns, numerous enough to keep the PE busy
            # (and the HAM clock ramping) through the ~13us operand latency.
            wu_w = persist.tile([P, P], bf16, tag="wu_w", name="wu_w")
            wu_ps = psum1.tile([P, P], fp32, tag="ps1", name="wu_ps")
            nc.gpsimd.memset(wu_w, 0)
            for _ in range(WARMUP):
                nc.tensor.matmul(wu_ps, wu_w, wu_w, start=True, stop=True)

            # --- input DMAs, big and few, all on the Sync HWDGE queue in
            # consumption order. x goes through a 3-deep pool; the first
            # three groups' loads have no WAR waits, later ones queue up
            # behind mm1 completions (by then Sync is idle).
            xg = {}

            def load_x(gi):
                g0, gs, _ = groups[gi]
                t = xpool.tile([P, KD, MM_FREE], bf16, tag="xg", name=f"xg_{gi}")
                nc.sync.dma_start(out=t[:, :, :gs], in_=xT[:, :, g0:g0 + gs])
                xg[gi] = t

            w1c = [[None] * NW1C for _ in range(n_sets)]
            w2c = [[None] * NW2Q for _ in range(n_sets)]

            def load_w1(s, c):
                t = persist.tile([P, KD, W1CH], bf16, tag=f"w1_{s}_{c}",
                                 name=f"w1_{s}_{c}")
                nc.sync.dma_start(
                    out=t, in_=w1d[s][:, :, c * W1CH:(c + 1) * W1CH])
                w1c[s][c] = t

            def load_w2(s, q):
                t = persist.tile([P, 4, D], bf16, tag=f"w2_{s}_{q}",
                                 name=f"w2_{s}_{q}")
                nc.sync.dma_start(out=t, in_=w2d[s][:, 4 * q:4 * (q + 1), :])
                w2c[s][q] = t

            load_x(0)
            load_w1(0, 0)
            load_w1(0, 1)
            wt_sb = persist.tile([P, NB], fp32, tag="wt", name="wt_sb")
            nc.sync.dma_start(out=wt_sb, in_=wt[:, :])
            for c in range(2, NW1C):
                load_w1(0, c)
            load_x(1)
            load_x(2)
            for q in range(NW2Q):
                load_w2(0, q)
            if n_sets > 1:
                for c in range(NW1C):
                    load_w1(1, c)
                for q in range(NW2Q):
                    load_w2(1, q)
            for gi in range(3, len(groups)):
                load_x(gi)

            # --- software-pipelined group schedule: mm1(g0), mm1(g1),
            # mm2(g0), mm1(g2), mm2(g1), ... so the PE always has
            # independent work at every mm1->mm2 boundary.
            def mm1(gi):
                g0, gs, s = groups[gi]
                ht = hpool.tile([P, KH, MM_FREE], bf16, tag="hT", name=f"hT_{g0}")
                for i in range(KH):
                    ci, co = divmod(i, W1CH // P)
                    ps = psum1.tile([P, MM_FREE], fp32, tag="ps1",
                                    name=f"ps1_{g0}_{i}")
                    for k in range(KD):
                        nc.tensor.matmul(
                            ps[:, :gs],
                            w1c[s][ci][:, k, co * P:(co + 1) * P],
                            xg[gi][:, k, :gs],
                            start=(k == 0),
                            stop=(k == KD - 1),
                        )
                    nc.scalar.activation(
                        ht[:, i, :gs], ps[:, :gs],
                        mybir.ActivationFunctionType.Silu,
                    )
                return ht

            def mm2(gi, ht):
                g0, gs, s = groups[gi]
                for t in range(-(-gs // P)):
                    tok = g0 + t * P
                    tn = min(P, gs - t * P)
                    for j in range(D // MM_FREE):
                        ps2 = psum2.tile([P, MM_FREE], fp32, tag="ps2",
                                         name=f"ps2_{tok}_{j}")
                        for i in range(KH):
                            nc.tensor.matmul(
                                ps2[:tn],
                                ht[:, i, t * P:t * P + tn],
                                w2c[s][i // 4][:, i % 4,
                                               j * MM_FREE:(j + 1) * MM_FREE],
                                start=(i == 0),
                                stop=(i == KH - 1),
                            )
                        ot = opool.tile([P, MM_FREE], bf16, tag="ot",
                                        name=f"ot_{tok}_{j}")
                        nc.vector.tensor_scalar_mul(
                            ot[:tn], ps2[:tn], wt_sb[:tn, tok // P:tok // P + 1])
                        nc.scalar.dma_start(
                            out=out[tok:tok + tn, j * MM_FREE:(j + 1) * MM_FREE],
                            in_=ot[:tn])

            prev = (0, mm1(0))
            for gi in range(1, len(groups)):
                ht = mm1(gi)
                mm2(*prev)
                prev = (gi, ht)
            mm2(*prev)

    nc.compile()
    return nc


def _get_compiled(C_A, C_B):
    key = (C_A, C_B)
    if key not in _compiled:
        _compiled[key] = _build(C_A, C_B)
    return _compiled[key]


def _plan_capacity(counts):
    """Pick (C_A, C_B). Total capacity C_A + C_B is what the PE pays for, so
    prefer the single-set layout (C_A = max expert load, C_B = 0: no second
    weight set, 8 MB/core less DMA) unless a secondary block genuinely
    lowers C. C_A need not be 128-aligned: mm1 cost is token-proportional
    and mm2 handles a partial final 128-block."""
    mean_cap = int(-(-counts.sum() // (NCORES * P)) * P)
    max_cap = int(counts.max())
    best = (max_cap, 0)
    for C_A in range(mean_cap, max_cap + P, P):
        if C_A + P >= max_cap:
            break
        over = np.maximum(counts - C_A, 0)
        nblocks = int(np.sum(-(-over // P)))
        if nblocks <= NCORES:
            best = (C_A, P)
            break
    return best


def _kpack(a, k):
    """[k*128, F] -> [128, k, F] partition-major contiguous."""
    f = a.shape[1]
    return np.ascontiguousarray(a.reshape(k, P, f).transpose(1, 0, 2))


def kernel(**inputs):
    x = np.asarray(inputs["x"], dtype=np.float32)
    Wg = np.asarray(inputs["Wg"], dtype=np.float32)
    W1 = np.asarray(inputs["W1"], dtype=np.float32)
    W2 = np.asarray(inputs["W2"], dtype=np.float32)
    xf = np.ascontiguousarray(x.reshape(-1, D))

    # --- host-side gate + top-2 routing (float64; ordering matches f32 ref) ---
    logits = xf.astype(np.float64) @ Wg.astype(np.float64)
    w = np.exp(logits - logits.max(axis=-1, keepdims=True))
    w /= w.sum(axis=-1, keepdims=True)
    order = np.argsort(-w, axis=-1, kind="stable")[:, :TOP_K]  # [N, 2] expert ids
    tw = np.take_along_axis(w, order, axis=-1)
    tw = tw / tw.sum(axis=-1, keepdims=True)  # renormalized combine weights

    counts = np.bincount(order.ravel(), minlength=E)
    C_A, C_B = _plan_capacity(counts)
    C = C_A + C_B

    nc = _get_compiled(C_A, C_B)

    # --- dispatch: primary segment per expert-owner core + overflow blocks ---
    bf = ml_dtypes.bfloat16
    tok_of = []    # per expert: token ids routed to it (ascending)
    wt_of = []     # matching combine weights
    for e in range(E):
        sel = np.nonzero((order == e).any(axis=-1))[0]
        slot = (order[sel, 1] == e).astype(np.int64)
        tok_of.append(sel)
        wt_of.append(tw[sel, slot].astype(np.float32))

    # overflow blocks (expert, token ids, weights), <=128 tokens each
    blocks = []
    for e in range(E):
        for b0 in range(C_A, len(tok_of[e]), P):
            blocks.append((e, tok_of[e][b0:b0 + P], wt_of[e][b0:b0 + P]))
    assert len(blocks) <= NCORES, (counts, C_A, C_B)

    pos = np.empty((N, TOP_K), dtype=np.int64)
    in_maps = []
    w1p = {}
    w2p = {}

    def packed_w(e):
        if e not in w1p:
            w1p[e] = _kpack(np.ascontiguousarray(W1[e]).astype(bf), KD)
            w2p[e] = _kpack(np.ascontiguousarray(W2[e]).astype(bf), KH)
        return w1p[e], w2p[e]

    for c in range(NCORES):
        prim_tok = tok_of[c][:C_A]
        prim_wt = wt_of[c][:C_A]
        slot = (order[prim_tok, 1] == c).astype(np.int64)
        pos[prim_tok, slot] = c * C + np.arange(len(prim_tok))

        xTe = np.zeros((D, C), dtype=bf)
        xTe[:, :len(prim_tok)] = xf[prim_tok].T.astype(bf)
        wtp = np.zeros(C, dtype=np.float32)
        wtp[:len(prim_tok)] = prim_wt

        a1, a2 = packed_w(c)
        m = {"w10": a1, "w20": a2}
        if C_B:
            if c < len(blocks):
                be, btok, bwt = blocks[c]
                xTe[:, C_A:C_A + len(btok)] = xf[btok].T.astype(bf)
                wtp[C_A:C_A + len(btok)] = bwt
                bslot = (order[btok, 1] == be).astype(np.int64)
                pos[btok, bslot] = c * C + C_A + np.arange(len(btok))
                b1, b2 = packed_w(be)
                m["w11"] = b1
                m["w21"] = b2
            else:
                m["w11"] = np.zeros((P, KD, H), dtype=bf)
                m["w21"] = np.zeros((P, KH, D), dtype=bf)
        m["xT"] = _kpack(xTe, KD)
        nb = -(-C // P)
        wtpad = np.zeros(nb * P, dtype=np.float32)
        wtpad[:C] = wtp
        m["wt"] = np.ascontiguousarray(wtpad.reshape(nb, P).T)
        in_maps.append(m)

    from concourse.bass_utils import run_bass_kernel_spmd

    # The SPMD launch reaches the 8 NeuronCores through jax/PJRT. If the
    # calling process pinned jax to CPU (e.g. to run the reference), flip to
    # the axon platform for the launch and restore afterwards.
    import jax

    flipped = False
    try:
        n_acc = len([d for d in jax.devices() if d.platform != "cpu"])
    except Exception:
        n_acc = 0

    def _clear_backends():
        try:
            import jax.extend.backend as jeb
            jeb.clear_backends()
        except Exception:
            from jax._src import xla_bridge
            xla_bridge._clear_backends()

    if n_acc < NCORES:
        prev = jax.config.jax_platforms
        jax.config.update("jax_platforms", "axon")
        _clear_backends()
        flipped = True
    try:
        res = run_bass_kernel_spmd(nc, in_maps, core_ids=list(range(NCORES)))
    finally:
        if flipped:
            jax.config.update("jax_platforms", prev)
            _clear_backends()

    Y = np.concatenate(
        [np.asarray(res.results[c]["out"], dtype=np.float32)
         for c in range(NCORES)], axis=0)
    outf = Y[pos[:, 0]] + Y[pos[:, 1]]
    return outf.reshape(B, T, D).astype(np.float32)
